# revision 1
# baseline (speedup 1.0000x reference)
"""Differentiable-JPEG forward kernel for 8 Trainium2 NeuronCores.

Strategy (pure data parallel, one image per core):
  RGB->YCbCr + offsets are folded into pass-A matmuls (lhsT = Mfwd[c,c']*BD(D^T),
  plus a K=1 rank-1 matmul for the +0.5 chroma offset). The 8x8 DCT is applied
  with 128x128 block-diagonal DCT matrices; the h<->w layout swap between the
  two DCT directions uses the DVE 32x32 StreamTranspose, which composes with
  the block-diagonal matmuls so only 32-blocked ("Z-layout") transposes are
  ever needed. The 20-sigmoid soft-round collapses to floor(x) + sigmoid(
  50*frac(x) - 25) (tail terms < 2e-11). Only the quantization *correction*
  E = IDCT(delta*qt) flows through the inverse passes (identity path:
  out = clip(X + Minv @ E)), so the post-sigmoid passes can use fp32r
  (11-fraction-bit) matmuls without precision risk; the forward DCT pass B
  stays fp32.
"""
import sys
sys.path.insert(0, '/opt/trn_rl_repo')
import numpy as np
from contextlib import ExitStack

H = W = 512
NCORES = 8
TEMP = 50.0
MAGIC = 12582912.0  # 1.5 * 2^23

MFWD = np.array([[0.299, 0.587, 0.114],
                 [-0.168736, -0.331264, 0.5],
                 [0.5, -0.418688, -0.081312]], dtype=np.float64)
OFFS = np.array([0.0, 0.5, 0.5], dtype=np.float64)
MINV = np.array([[1.0, 0.0, 1.402],
                 [1.0, -0.344136, -0.714136],
                 [1.0, 1.772, 0.0]], dtype=np.float64)


def _dct8():
    n = np.arange(8)
    mat = np.sqrt(2.0 / 8) * np.cos((2 * n[None, :] + 1) * n[:, None] * np.pi / 16.0)
    mat[0, :] = np.sqrt(1.0 / 8)
    return mat


_D8 = _dct8()
_BD = np.kron(np.eye(16), _D8)       # 128x128, block diag of D
_BDT = _BD.T.copy()

_MODULE_CACHE = {}


def _build_module(reps=1):
    import concourse.bass as bass
    import concourse.tile as tile
    from concourse import bacc, mybir

    F32 = mybir.dt.float32
    F32R = mybir.dt.float32r
    ALU = mybir.AluOpType
    AF = mybir.ActivationFunctionType

    nc = bacc.Bacc("TRN2", target_bir_lowering=False, debug=False,
                   num_devices=NCORES)

    img_d = nc.dram_tensor("img", [3, H, W], F32, kind="ExternalInput").ap()
    wa_d = nc.dram_tensor("wa", [128, 9 * 128], F32, kind="ExternalInput").ap()
    wb_d = nc.dram_tensor("wb", [128, 128], F32, kind="ExternalInput").ap()
    wc_d = nc.dram_tensor("wc", [128, 128], F32, kind="ExternalInput").ap()
    wd_d = nc.dram_tensor("wd", [128, 10 * 128], F32, kind="ExternalInput").ap()
    offa_d = nc.dram_tensor("offa", [1, 128], F32, kind="ExternalInput").ap()
    ones_d = nc.dram_tensor("ones", [1, 512], F32, kind="ExternalInput").ap()
    qtinv_d = nc.dram_tensor("qtinv", [128, 1536], F32, kind="ExternalInput").ap()
    qt_d = nc.dram_tensor("qt", [128, 1536], F32, kind="ExternalInput").ap()
    out_d = nc.dram_tensor("out", [3, H, W], F32, kind="ExternalOutput").ap()

    with tile.TileContext(nc) as tc, ExitStack() as ctx:
        const = ctx.enter_context(tc.tile_pool(name="const", bufs=1))
        imgp = ctx.enter_context(tc.tile_pool(name="imgp", bufs=12))
        sb_z = ctx.enter_context(tc.tile_pool(name="sb_z", bufs=6))
        sb_q = ctx.enter_context(tc.tile_pool(name="sb_q", bufs=6))
        sb_g = ctx.enter_context(tc.tile_pool(name="sb_g", bufs=7))
        sb_u = ctx.enter_context(tc.tile_pool(name="sb_u", bufs=4))
        sb_o = ctx.enter_context(tc.tile_pool(name="sb_o", bufs=4))
        pa = ctx.enter_context(tc.tile_pool(name="pa", bufs=2, space="PSUM"))
        pb = ctx.enter_context(tc.tile_pool(name="pb", bufs=2, space="PSUM"))
        pc = ctx.enter_context(tc.tile_pool(name="pc", bufs=2, space="PSUM"))
        pd = ctx.enter_context(tc.tile_pool(name="pd", bufs=2, space="PSUM"))

        # pass-A constants + the first image tiles go first so the PE can
        # start while the bulky later-stage constants stream in behind them.
        wa = const.tile([128, 9 * 128], F32R, tag="wa")
        nc.sync.dma_start(wa[:], wa_d.bitcast(F32R))
        offa = const.tile([1, 128], F32R, tag="offa")
        nc.sync.dma_start(offa[:], offa_d.bitcast(F32R))
        ones = const.tile([1, 512], F32R, tag="ones")
        nc.sync.dma_start(ones[:], ones_d.bitcast(F32R))
        bm25 = const.tile([128, 1], F32, tag="bm25")
        nc.vector.memset(bm25[:], -25.0)

        img0 = []
        for c in range(3):
            im = imgp.tile([128, 512], F32R, tag="img")
            nc.sync.dma_start(im[:], img_d[c, 0:128, :].bitcast(F32R))
            img0.append(im)

        wb = const.tile([128, 128], F32, tag="wb")
        nc.sync.dma_start(wb[:], wb_d)
        qtinv = const.tile([128, 1536], F32, tag="qtinv")
        nc.sync.dma_start(qtinv[:], qtinv_d)
        qt = const.tile([128, 1536], F32, tag="qt")
        nc.sync.dma_start(qt[:], qt_d)
        wc = const.tile([128, 128], F32R, tag="wc")
        nc.sync.dma_start(wc[:], wc_d.bitcast(F32R))
        wd = const.tile([128, 10 * 128], F32R, tag="wd")
        nc.sync.dma_start(wd[:], wd_d.bitcast(F32R))

        pending = None
        for rep in range(reps):
            for t in range(4):
                hs = slice(t * 128, (t + 1) * 128)
                if rep == 0 and t == 0:
                    imgs = img0
                else:
                    imgs = []
                    for c in range(3):
                        im = imgp.tile([128, 512], F32R, tag="img")
                        nc.sync.dma_start(im[:], img_d[c, hs, :].bitcast(F32R))
                        imgs.append(im)

                # per-channel: A -> ST1 -> B -> quant chain -> gg
                urs = []
                for c in range(3):
                    csl = slice(c * 512, (c + 1) * 512)
                    pA = pa.tile([128, 512], F32, tag="pa")
                    for c2 in range(3):
                        nc.tensor.matmul(pA[:], wa[:, (3 * c + c2) * 128:
                                                    (3 * c + c2 + 1) * 128],
                                         imgs[c2][:],
                                         start=(c2 == 0),
                                         stop=(c2 == 2 and c == 0))
                    if c != 0:
                        nc.tensor.matmul(pA[:], offa[:], ones[:],
                                         start=False, stop=True)
                    z1 = sb_z.tile([128, 512], F32, tag="z1")
                    nc.vector.transpose(z1[:], pA[:])
                    pB = pb.tile([128, 512], F32, tag="pb")
                    nc.tensor.matmul(pB[:], wb[:], z1[:],
                                     start=True, stop=True)
                    xt = sb_q.tile([128, 512], F32, tag="xt")
                    tt = sb_q.tile([128, 512], F32, tag="tt")
                    nm = sb_q.tile([128, 512], F32, tag="nm")
                    ss = sb_q.tile([128, 512], F32, tag="ss")
                    dd = sb_q.tile([128, 512], F32, tag="dd")
                    gg = sb_g.tile([128, 512], F32R, tag="gg")
                    # soft-round correction in half-tiles so each stage can
                    # start as soon as the first half of its input is ready
                    for hh in range(2):
                        sl = slice(hh * 256, (hh + 1) * 256)
                        qsl = slice(c * 512 + hh * 256,
                                    c * 512 + (hh + 1) * 256)
                        nc.vector.tensor_tensor(xt[:, sl], pB[:, sl],
                                                qtinv[:, qsl], op=ALU.mult)
                        nc.gpsimd.tensor_scalar(tt[:, sl], xt[:, sl], -0.5,
                                                MAGIC, op0=ALU.add,
                                                op1=ALU.add)
                        nc.vector.scalar_tensor_tensor(nm[:, sl], tt[:, sl],
                                                       MAGIC, xt[:, sl],
                                                       op0=ALU.subtract,
                                                       op1=ALU.subtract)
                        nc.scalar.activation(ss[:, sl], nm[:, sl], AF.Sigmoid,
                                             bias=bm25[:], scale=-TEMP)
                        nc.gpsimd.tensor_tensor(dd[:, sl], ss[:, sl],
                                                nm[:, sl], op=ALU.add)
                        nc.gpsimd.tensor_tensor(gg[:, sl], dd[:, sl],
                                                qt[:, qsl], op=ALU.mult)

                    urs.append(gg)

                def tail_phase(ggs=urs, imgs_t=imgs, hs_t=hs):
                    urs2 = []
                    for c in range(3):
                        pC = pc.tile([128, 512], F32, tag="pc")
                        nc.tensor.matmul(pC[:], wc[:], ggs[c][:],
                                         start=True, stop=True)
                        u = sb_u.tile([128, 512], F32, tag="u")
                        nc.vector.transpose(u[:], pC[:])
                        ur = sb_u.tile([128, 512], F32R, tag="ur")
                        nc.scalar.copy(ur[:], u[:])
                        urs2.append(ur)
                    for c in range(3):
                        pD = pd.tile([128, 512], F32, tag="pd")
                        nz = [c2 for c2 in range(3) if MINV[c, c2] != 0.0]
                        for j, c2 in enumerate(nz):
                            nc.tensor.matmul(pD[:], wd[:, (3 * c + c2) * 128:
                                                        (3 * c + c2 + 1) * 128],
                                             urs2[c2][:],
                                             start=(j == 0), stop=False)
                        nc.tensor.matmul(pD[:], wd[:, 9 * 128:10 * 128],
                                         imgs_t[c][:], start=False, stop=True)
                        o = sb_o.tile([128, 512], F32, tag="o")
                        nc.vector.tensor_scalar(o[:], pD[:], 0.0, 1.0,
                                                op0=ALU.max, op1=ALU.min)
                        nc.scalar.dma_start(out_d[c, hs_t, :], o[:])
                if pending is not None:
                    pending()
                pending = tail_phase

        if pending is not None:
            pending()

    nc.compile()
    return nc


def _host_arrays(q_y, q_c):
    qy = np.clip(q_y.astype(np.float64), 2.0, 15.0)
    qc = np.clip(q_c.astype(np.float64), 2.0, 15.0)
    qts = [qy, qc, qc]

    wa = np.zeros((128, 9 * 128), np.float32)
    wd = np.zeros((128, 10 * 128), np.float32)
    for c in range(3):
        for c2 in range(3):
            wa[:, (3 * c + c2) * 128:(3 * c + c2 + 1) * 128] = \
                (MFWD[c, c2] * _BDT).astype(np.float32)
            wd[:, (3 * c + c2) * 128:(3 * c + c2 + 1) * 128] = \
                (MINV[c, c2] * _BD).astype(np.float32)
    wd[:, 9 * 128:] = np.eye(128, dtype=np.float32)
    wb = _BD.astype(np.float32)
    wc = _BDT.astype(np.float32)
    offa = (0.5 * np.sqrt(8.0) *
            (np.arange(128) % 8 == 0)).astype(np.float32)[None, :]

    p_idx = np.arange(128) % 8
    f_idx = np.arange(512) % 8
    qtinv = np.zeros((128, 1536), np.float32)
    qt = np.zeros((128, 1536), np.float32)
    for c in range(3):
        rep = qts[c][np.ix_(f_idx, p_idx)].T
        qt[:, c * 512:(c + 1) * 512] = rep.astype(np.float32)
        qtinv[:, c * 512:(c + 1) * 512] = (1.0 / rep).astype(np.float32)
    return dict(wa=wa, wb=wb, wc=wc, wd=wd, offa=offa, qtinv=qtinv, qt=qt,
                ones=np.ones((1, 512), np.float32))


class _Runner:
    """Jit the bass program once; later calls only ship data."""

    def __init__(self, nc):
        import jax
        from jax.sharding import Mesh, PartitionSpec
        from jax.experimental.shard_map import shard_map
        from concourse import bass2jax, mybir
        from concourse.bass2jax import _bass_exec_p, install_neuronx_cc_hook

        install_neuronx_cc_hook()
        self.jax = jax
        pname = nc.partition_id_tensor.name if nc.partition_id_tensor else None
        in_names, out_names, out_avals, zero_outs = [], [], [], []
        for alloc in nc.m.functions[0].allocations:
            if not isinstance(alloc, mybir.MemoryLocationSet):
                continue
            name = alloc.memorylocations[0].name
            if alloc.kind == "ExternalInput":
                if name != pname:
                    in_names.append(name)
            elif alloc.kind == "ExternalOutput":
                out_names.append(name)
                shape = tuple(alloc.tensor_shape)
                dtype = mybir.dt.np(alloc.dtype)
                out_avals.append(jax.core.ShapedArray(shape, dtype))
                zero_outs.append(np.zeros(shape, dtype))
        self.in_names, self.out_names = in_names, out_names
        self.out_shapes = [z.shape for z in zero_outs]
        # outputs are fully overwritten by the kernel; ship the placeholder
        # buffers to the device once instead of 25MB per call
        self.zeros = [jax.device_put(
            np.zeros((NCORES * z.shape[0], *z.shape[1:]), z.dtype))
            for z in zero_outs]
        self._const_cache = {}
        all_names = list(in_names) + list(out_names)
        if pname:
            all_names.append(pname)

        def _body(*args):
            operands = list(args)
            if pname:
                operands.append(bass2jax.partition_id_tensor())
            return tuple(_bass_exec_p.bind(
                *operands,
                out_avals=tuple(out_avals),
                in_names=tuple(all_names),
                out_names=tuple(out_names),
                lowering_input_output_aliases=(),
                sim_require_finite=True,
                sim_require_nnan=True,
                nc=nc,
            ))

        devices = jax.devices()[:NCORES]
        mesh = Mesh(np.asarray(devices), ("core",))
        n_ops = len(in_names) + len(zero_outs)
        self.fn = jax.jit(
            shard_map(_body, mesh=mesh,
                      in_specs=(PartitionSpec("core"),) * n_ops,
                      out_specs=(PartitionSpec("core"),) * len(out_names),
                      check_rep=False),
            keep_unused=True)

    def __call__(self, in_maps):
        concat = []
        for n in self.in_names:
            arrs = [np.asarray(in_maps[c][n]) for c in range(NCORES)]
            if all(a is arrs[0] for a in arrs[1:]):
                # replicated constant (weights/tables): cache on device
                key = (n, arrs[0].tobytes())
                dev = self._const_cache.get(key)
                if dev is None:
                    self._const_cache.clear()
                    dev = self.jax.device_put(
                        np.concatenate(arrs, axis=0))
                    self._const_cache[key] = dev
                concat.append(dev)
            else:
                concat.append(np.concatenate(arrs, axis=0))
        outs = self.fn(*concat, *self.zeros)
        self.jax.block_until_ready(outs)
        return [{n: np.asarray(outs[i]).reshape(NCORES, *self.out_shapes[i])[c]
                 for i, n in enumerate(self.out_names)}
                for c in range(NCORES)]


def kernel(images, q_y, q_c, _reps=1, _time_only=False):
    images = np.ascontiguousarray(np.asarray(images, dtype=np.float32))
    shared = _host_arrays(np.asarray(q_y), np.asarray(q_c))

    if _reps not in _MODULE_CACHE:
        _MODULE_CACHE[_reps] = _Runner(_build_module(reps=_reps))
    run = _MODULE_CACHE[_reps]

    in_maps = [dict(img=np.ascontiguousarray(images[i]), **shared)
               for i in range(NCORES)]
    results = run(in_maps)
    if _time_only:
        return None
    out = np.stack([results[i]["out"] for i in range(NCORES)], axis=0)
    return out.astype(np.float32)



# revision 11
# speedup vs baseline: 5447.6543x; 5447.6543x over previous
"""Differentiable-JPEG forward kernel for 8 Trainium2 NeuronCores.

Strategy (pure data parallel, one image per core):
  RGB->YCbCr + offsets are folded into pass-A matmuls (lhsT = Mfwd[c,c']*BD(D^T),
  plus a K=1 rank-1 matmul for the +0.5 chroma offset). The 8x8 DCT is applied
  with 128x128 block-diagonal DCT matrices; the h<->w layout swap between the
  two DCT directions uses the DVE 32x32 StreamTranspose, which composes with
  the block-diagonal matmuls so only 32-blocked ("Z-layout") transposes are
  ever needed. The 20-sigmoid soft-round collapses to floor(x) + sigmoid(
  50*frac(x) - 25) (tail terms < 2e-11).

  Engine budget (per core, per image):
    PE    : all 2D-DCT passes in fp32r (1 cycle/row), identity +X path
    DVE   : 2 stream-transposes + one fused custom op f = frac(pB*qtinv)
            (magic-number round inside the 8-stage DVE datapath)
    Act   : ss = sigmoid(50 f - 25), and clip via two Relu(1-x) passes
    Pool  : dd = ss - f, gg = dd * qt
    SP    : all DMA issues (input tiles, output tiles, constants)
"""
import sys
sys.path.insert(0, '/opt/trn_rl_repo')
import numpy as np
from contextlib import ExitStack

H = W = 512
NCORES = 8
TEMP = 50.0
MAGIC = 12582912.0  # 1.5 * 2^23

MFWD = np.array([[0.299, 0.587, 0.114],
                 [-0.168736, -0.331264, 0.5],
                 [0.5, -0.418688, -0.081312]], dtype=np.float64)
OFFS = np.array([0.0, 0.5, 0.5], dtype=np.float64)
MINV = np.array([[1.0, 0.0, 1.402],
                 [1.0, -0.344136, -0.714136],
                 [1.0, 1.772, 0.0]], dtype=np.float64)


def _dct8():
    n = np.arange(8)
    mat = np.sqrt(2.0 / 8) * np.cos((2 * n[None, :] + 1) * n[:, None] * np.pi / 16.0)
    mat[0, :] = np.sqrt(1.0 / 8)
    return mat


_D8 = _dct8()
_BD = np.kron(np.eye(16), _D8)       # 128x128, block diag of D
_BDT = _BD.T.copy()

_MODULE_CACHE = {}
_DVE_OP = None


def _frac_mul_ref(in0, in1, s0, s1, imm2):
    m = (np.asarray(in0, np.float32) * np.asarray(in1, np.float32)).astype(np.float32)
    t1 = (m - np.float32(imm2)).astype(np.float32)
    t = (t1 + np.float32(s0)).astype(np.float32)
    fl = (t - np.float32(s1)).astype(np.float32)
    return (m - fl).astype(np.float32)


def _register_dve_op():
    """Register FRAC_MUL_ANT: out = frac(in0*in1) via magic-number rounding.
    5 ALU stages: m = a*b; t = (m - 0.5) + MAGIC; fl = t - MAGIC; out = m - fl.
    The -0.5 must be its own stage: MAGIC-0.5 is not representable in fp32."""
    global _DVE_OP
    if _DVE_OP is not None:
        return _DVE_OP
    from concourse import dve_ops as dops
    from concourse.dve_spec import Spec, Src0, Src1, C0, C1, C2, lower
    from concourse.dve_uop import DveOpSpec
    from concourse.dve_table_gen import dve_ver_for

    name = "FRAC_MUL_ANT"
    if name in dops._SUB_OPCODE_FOR_NAME:
        _DVE_OP = next(op for op in dops.OPS if op.name == name)
        return _DVE_OP

    m = Src0 * Src1
    fl = ((m - C2) + C0) - C1
    spec = Spec(body=m - fl, reference=_frac_mul_ref)
    ver = dve_ver_for("TRN2")
    opcode = max(dops._SUB_OPCODE_FOR_NAME.values()) + 1
    tmp = DveOpSpec(name=name, opcode=opcode, uops=lower(spec, ver=ver),
                    rd1_en=dops.has_src1(spec))
    op = dops.DveOp(name, spec, subdim=False, uops_sha={ver: tmp.sha(ver)})
    dops.OPS.append(op)
    dops.CUSTOM_DVE_SPECS[name] = spec
    dops._SUB_OPCODE_FOR_NAME[name] = opcode
    _DVE_OP = op
    return op


def _build_module(reps=1):
    import concourse.bass as bass
    import concourse.tile as tile
    from concourse import bacc, mybir

    F32 = mybir.dt.float32
    F32R = mybir.dt.float32r
    ALU = mybir.AluOpType
    AF = mybir.ActivationFunctionType

    dve_op = _register_dve_op()

    nc = bacc.Bacc("TRN2", target_bir_lowering=False, debug=False,
                   num_devices=NCORES)

    img_d = nc.dram_tensor("img", [3, H, W], F32, kind="ExternalInput").ap()
    wa_d = nc.dram_tensor("wa", [128, 9 * 128], F32, kind="ExternalInput").ap()
    wb_d = nc.dram_tensor("wb", [128, 128], F32, kind="ExternalInput").ap()
    wc_d = nc.dram_tensor("wc", [128, 128], F32, kind="ExternalInput").ap()
    wd_d = nc.dram_tensor("wd", [128, 10 * 128], F32, kind="ExternalInput").ap()
    offa_d = nc.dram_tensor("offa", [1, 128], F32, kind="ExternalInput").ap()
    ones_d = nc.dram_tensor("ones", [1, 512], F32, kind="ExternalInput").ap()
    qtinv_d = nc.dram_tensor("qtinv", [128, 1536], F32, kind="ExternalInput").ap()
    qt_d = nc.dram_tensor("qt", [128, 1536], F32, kind="ExternalInput").ap()
    out_d = nc.dram_tensor("out", [3, H, W], F32, kind="ExternalOutput").ap()

    with tile.TileContext(nc) as tc, ExitStack() as ctx:
        const = ctx.enter_context(tc.tile_pool(name="const", bufs=1))
        imgp = ctx.enter_context(tc.tile_pool(name="imgp", bufs=12))
        sb_z = ctx.enter_context(tc.tile_pool(name="sb_z", bufs=4))
        sb_f = ctx.enter_context(tc.tile_pool(name="sb_f", bufs=6))
        sb_s = ctx.enter_context(tc.tile_pool(name="sb_s", bufs=4))
        sb_d = ctx.enter_context(tc.tile_pool(name="sb_d", bufs=4))
        sb_g = ctx.enter_context(tc.tile_pool(name="sb_g", bufs=6))
        sb_u = ctx.enter_context(tc.tile_pool(name="sb_u", bufs=7))
        sb_o = ctx.enter_context(tc.tile_pool(name="sb_o", bufs=6))
        pa = ctx.enter_context(tc.tile_pool(name="pa", bufs=2, space="PSUM"))
        pb = ctx.enter_context(tc.tile_pool(name="pb", bufs=2, space="PSUM"))
        pc = ctx.enter_context(tc.tile_pool(name="pc", bufs=2, space="PSUM"))
        pd = ctx.enter_context(tc.tile_pool(name="pd", bufs=2, space="PSUM"))

        # pass-A constants + the first image tiles go first so the PE can
        # start while the bulky later-stage constants stream in behind them.
        wa = const.tile([128, 9 * 128], F32R, tag="wa")
        nc.sync.dma_start(wa[:], wa_d.bitcast(F32R))
        offa = const.tile([1, 128], F32R, tag="offa")
        nc.sync.dma_start(offa[:], offa_d.bitcast(F32R))
        ones = const.tile([1, 512], F32R, tag="ones")
        nc.sync.dma_start(ones[:], ones_d.bitcast(F32R))
        bm25 = const.tile([128, 1], F32, tag="bm25")
        nc.vector.memset(bm25[:], -25.0)
        bp1 = const.tile([128, 1], F32, tag="bp1")
        nc.vector.memset(bp1[:], 1.0)

        img0 = []
        for c in range(3):
            im = imgp.tile([128, 512], F32R, tag="img")
            nc.sync.dma_start(im[:], img_d[c, 0:128, :].bitcast(F32R))
            img0.append(im)

        wb = const.tile([128, 128], F32R, tag="wb")
        nc.sync.dma_start(wb[:], wb_d.bitcast(F32R))
        qtinv = const.tile([128, 1536], F32, tag="qtinv")
        nc.sync.dma_start(qtinv[:], qtinv_d)
        qt = const.tile([128, 1536], F32, tag="qt")
        nc.sync.dma_start(qt[:], qt_d)
        wc = const.tile([128, 128], F32R, tag="wc")
        nc.sync.dma_start(wc[:], wc_d.bitcast(F32R))
        wd = const.tile([128, 10 * 128], F32R, tag="wd")
        nc.sync.dma_start(wd[:], wd_d.bitcast(F32R))

        pending = None
        for rep in range(reps):
            for t in range(4):
                hs = slice(t * 128, (t + 1) * 128)
                if rep == 0 and t == 0:
                    imgs = img0
                else:
                    imgs = []
                    for c in range(3):
                        im = imgp.tile([128, 512], F32R, tag="img")
                        nc.sync.dma_start(im[:], img_d[c, hs, :].bitcast(F32R))
                        imgs.append(im)

                # forward: A -> T -> B -> quant chain
                ggs = []
                for c in range(3):
                    csl = slice(c * 512, (c + 1) * 512)
                    pA = pa.tile([128, 512], F32, tag="pa")
                    for c2 in range(3):
                        nc.tensor.matmul(pA[:], wa[:, (3 * c + c2) * 128:
                                                    (3 * c + c2 + 1) * 128],
                                         imgs[c2][:],
                                         start=(c2 == 0),
                                         stop=(c2 == 2 and c == 0))
                    if c != 0:
                        nc.tensor.matmul(pA[:], offa[:], ones[:],
                                         start=False, stop=True)
                    z1 = sb_z.tile([128, 512], F32, tag="z1")
                    nc.vector.transpose(z1[:], pA[:])
                    z1r = sb_z.tile([128, 512], F32R, tag="z1r")
                    nc.scalar.copy(z1r[:], z1[:])
                    pB = pb.tile([128, 512], F32, tag="pb")
                    nc.tensor.matmul(pB[:], wb[:], z1r[:],
                                     start=True, stop=True)
                    ff = sb_f.tile([128, 512], F32, tag="ff")
                    nc.vector._custom_dve(dve_op, out=ff[:], in0=pB[:],
                                          in1=qtinv[:, csl],
                                          s0=MAGIC, s1=MAGIC, imm2=0.5)
                    ss = sb_s.tile([128, 512], F32, tag="ss")
                    nc.scalar.activation(ss[:], ff[:], AF.Sigmoid,
                                         bias=bm25[:], scale=TEMP)
                    dd = sb_d.tile([128, 512], F32, tag="dd")
                    nc.gpsimd.tensor_tensor(dd[:], ss[:], ff[:],
                                            op=ALU.subtract)
                    gg = sb_g.tile([128, 512], F32R, tag="gg")
                    nc.gpsimd.tensor_tensor(gg[:], dd[:], qt[:, csl],
                                            op=ALU.mult)
                    ggs.append(gg)

                def tail_phase(ggs_t=ggs, imgs_t=imgs, hs_t=hs):
                    urs = []
                    for c in range(3):
                        pC = pc.tile([128, 512], F32, tag="pc")
                        nc.tensor.matmul(pC[:], wc[:], ggs_t[c][:],
                                         start=True, stop=True)
                        u = sb_u.tile([128, 512], F32, tag="u")
                        nc.vector.transpose(u[:], pC[:])
                        ur = sb_u.tile([128, 512], F32R, tag="ur")
                        nc.scalar.copy(ur[:], u[:])
                        urs.append(ur)
                    for c in range(3):
                        pD = pd.tile([128, 512], F32, tag="pd")
                        nz = [c2 for c2 in range(3) if MINV[c, c2] != 0.0]
                        for j, c2 in enumerate(nz):
                            nc.tensor.matmul(pD[:], wd[:, (3 * c + c2) * 128:
                                                        (3 * c + c2 + 1) * 128],
                                             urs[c2][:],
                                             start=(j == 0), stop=False)
                        nc.tensor.matmul(pD[:], wd[:, 9 * 128:10 * 128],
                                         imgs_t[c][:], start=False, stop=True)
                        o = sb_o.tile([128, 512], F32, tag="o")
                        nc.vector.tensor_scalar(o[:], pD[:], 0.0, 1.0,
                                                op0=ALU.max, op1=ALU.min)
                        nc.sync.dma_start(out_d[c, hs_t, :], o[:])
                if pending is not None:
                    pending()
                pending = tail_phase

        if pending is not None:
            pending()

    nc.compile()
    return nc


def _host_arrays(q_y, q_c):
    qy = np.clip(q_y.astype(np.float64), 2.0, 15.0)
    qc = np.clip(q_c.astype(np.float64), 2.0, 15.0)
    qts = [qy, qc, qc]

    wa = np.zeros((128, 9 * 128), np.float32)
    wd = np.zeros((128, 10 * 128), np.float32)
    for c in range(3):
        for c2 in range(3):
            wa[:, (3 * c + c2) * 128:(3 * c + c2 + 1) * 128] = \
                (MFWD[c, c2] * _BDT).astype(np.float32)
            wd[:, (3 * c + c2) * 128:(3 * c + c2 + 1) * 128] = \
                (MINV[c, c2] * _BD).astype(np.float32)
    wd[:, 9 * 128:] = np.eye(128, dtype=np.float32)
    wb = _BD.astype(np.float32)
    wc = _BDT.astype(np.float32)
    offa = (0.5 * np.sqrt(8.0) *
            (np.arange(128) % 8 == 0)).astype(np.float32)[None, :]

    p_idx = np.arange(128) % 8
    f_idx = np.arange(512) % 8
    qtinv = np.zeros((128, 1536), np.float32)
    qt = np.zeros((128, 1536), np.float32)
    for c in range(3):
        rep = qts[c][np.ix_(f_idx, p_idx)].T
        qt[:, c * 512:(c + 1) * 512] = rep.astype(np.float32)
        qtinv[:, c * 512:(c + 1) * 512] = (1.0 / rep).astype(np.float32)
    return dict(wa=wa, wb=wb, wc=wc, wd=wd, offa=offa, qtinv=qtinv, qt=qt,
                ones=np.ones((1, 512), np.float32))


class _Runner:
    """Jit the bass program once; later calls only ship data."""

    def __init__(self, nc):
        import jax
        from jax.sharding import Mesh, PartitionSpec
        from jax.experimental.shard_map import shard_map
        from concourse import bass2jax, mybir
        from concourse.bass2jax import _bass_exec_p, install_neuronx_cc_hook

        install_neuronx_cc_hook()
        self.jax = jax
        pname = nc.partition_id_tensor.name if nc.partition_id_tensor else None
        in_names, out_names, out_avals, zero_outs = [], [], [], []
        for alloc in nc.m.functions[0].allocations:
            if not isinstance(alloc, mybir.MemoryLocationSet):
                continue
            name = alloc.memorylocations[0].name
            if alloc.kind == "ExternalInput":
                if name != pname:
                    in_names.append(name)
            elif alloc.kind == "ExternalOutput":
                out_names.append(name)
                shape = tuple(alloc.tensor_shape)
                dtype = mybir.dt.np(alloc.dtype)
                out_avals.append(jax.core.ShapedArray(shape, dtype))
                zero_outs.append(np.zeros(shape, dtype))
        self.in_names, self.out_names = in_names, out_names
        self.out_shapes = [z.shape for z in zero_outs]
        # outputs are fully overwritten by the kernel; ship the placeholder
        # buffers to the device once instead of 25MB per call
        self.zeros = [jax.device_put(
            np.zeros((NCORES * z.shape[0], *z.shape[1:]), z.dtype))
            for z in zero_outs]
        self._const_cache = {}
        all_names = list(in_names) + list(out_names)
        if pname:
            all_names.append(pname)

        def _body(*args):
            operands = list(args)
            if pname:
                operands.append(bass2jax.partition_id_tensor())
            return tuple(_bass_exec_p.bind(
                *operands,
                out_avals=tuple(out_avals),
                in_names=tuple(all_names),
                out_names=tuple(out_names),
                lowering_input_output_aliases=(),
                sim_require_finite=True,
                sim_require_nnan=True,
                nc=nc,
            ))

        devices = jax.devices()[:NCORES]
        mesh = Mesh(np.asarray(devices), ("core",))
        n_ops = len(in_names) + len(zero_outs)
        self.fn = jax.jit(
            shard_map(_body, mesh=mesh,
                      in_specs=(PartitionSpec("core"),) * n_ops,
                      out_specs=(PartitionSpec("core"),) * len(out_names),
                      check_rep=False),
            keep_unused=True)

    def __call__(self, in_maps):
        concat = []
        for n in self.in_names:
            arrs = [np.asarray(in_maps[c][n]) for c in range(NCORES)]
            if all(a is arrs[0] for a in arrs[1:]):
                # replicated constant (weights/tables): cache on device
                key = (n, arrs[0].tobytes())
                dev = self._const_cache.get(key)
                if dev is None:
                    self._const_cache.clear()
                    dev = self.jax.device_put(
                        np.concatenate(arrs, axis=0))
                    self._const_cache[key] = dev
                concat.append(dev)
            else:
                concat.append(np.concatenate(arrs, axis=0))
        outs = self.fn(*concat, *self.zeros)
        self.jax.block_until_ready(outs)
        return [{n: np.asarray(outs[i]).reshape(NCORES, *self.out_shapes[i])[c]
                 for i, n in enumerate(self.out_names)}
                for c in range(NCORES)]


def kernel(images, q_y, q_c, _reps=1, _time_only=False):
    images = np.ascontiguousarray(np.asarray(images, dtype=np.float32))
    shared = _host_arrays(np.asarray(q_y), np.asarray(q_c))

    if _reps not in _MODULE_CACHE:
        _MODULE_CACHE[_reps] = _Runner(_build_module(reps=_reps))
    run = _MODULE_CACHE[_reps]

    in_maps = [dict(img=np.ascontiguousarray(images[i]), **shared)
               for i in range(NCORES)]
    results = run(in_maps)
    if _time_only:
        return None
    out = np.stack([results[i]["out"] for i in range(NCORES)], axis=0)
    return out.astype(np.float32)
